# revision 1
# baseline (speedup 1.0000x reference)
"""Multi-head causal attention (B=4, T=2048, C=768, H=12, HS=64) on 8 trn2 cores.

v2: p-outer schedule with hand-interleaved filler so the ACT-bound attention
stream overlaps all projection GEMMs.

Sharding: 48 (batch, head) units -> 6 per core. Core c: batch c//2, heads
6*(c%2) .. 6*(c%2)+6. Each core computes a partial output projection
y_partial[T, C]; host sums the two partials per batch and adds the bias.

Layout (per core):
  xT      [C, T] bf16 input, pre-transposed on host
  pairQ/pairK[p] [128, T] bf16: partitions 0:64 head 2p, 64:128 head 2p+1,
          written directly from the QK matmul psum (weights pair-stacked).
  scores  st[tk, e, tq] psum f32 = matmul(lhsT=KT, rhs=QT) with the two
          heads e=0/1 row-tiled at tile_position (0,0)/(64,0) (concurrent).
  softmax no max-subtraction; exp on ACT with scale=1/8; ones-column in V
          accumulates the denominator as otu row 64.
  vaug    [128, tt, h, 65] bf16 (col 64 = 1.0)
  P@V     otu[65, e, tq] psum = matmul(lhsT=vaug, rhs=exp(st)) over tk tiles
  norm    reciprocal of row 64, broadcast via K=1 matmul (rb), multiply;
          rb/mul issued one chunk later so the PE never stalls on the chain.
  proj    y[tq, :] = sum_g matmul(lhsT=otn[:, g, tq], rhs=WpT[g])
"""

import numpy as np
import ml_dtypes

import concourse.bacc as bacc
import concourse.bass as bass
import concourse.tile as tile
from concourse import mybir
from concourse import bass_utils

B, T, C = 4, 2048, 768
H, HS = 12, 64
HL = 6            # heads per core
NCT = C // 128    # 6 contraction tiles
NTT = T // 128    # 16 t tiles
NTC = T // 512    # 4 t chunks
SCALE = 1.0 / 8.0  # 1/sqrt(HS)

F32 = mybir.dt.float32
BF16 = mybir.dt.bfloat16


def build_kernel(nc, repeat=1):
    xT = nc.dram_tensor("xT", [C, T], BF16, kind="ExternalInput").ap()
    # wq/wk: per pair p, [Wx_{2p} | Wx_{2p+1}] stacked on out cols -> psum
    # rows 0:64 head 2p, 64:128 head 2p+1 (the pairQ/pairK layout).
    wq = nc.dram_tensor("wq", [3, NCT, 128, 128], BF16, kind="ExternalInput").ap()
    wk = nc.dram_tensor("wk", [3, NCT, 128, 128], BF16, kind="ExternalInput").ap()
    wv = nc.dram_tensor("wv", [NCT, 128, HL * HS], BF16, kind="ExternalInput").ap()
    wpt = nc.dram_tensor("wpt", [3, 128, C], BF16, kind="ExternalInput").ap()
    y = nc.dram_tensor("y", [T, C], BF16, kind="ExternalOutput").ap()

    with tile.TileContext(nc) as tc:
        with (
            tc.tile_pool(name="consts", bufs=1) as consts,
            tc.tile_pool(name="xw", bufs=1) as xw,
            tc.tile_pool(name="pt", bufs=16) as ptp,
            tc.tile_pool(name="small", bufs=4) as small,
            tc.tile_pool(name="ysb", bufs=3) as ysbp,
            # PSUM budget (8 banks): st 2x2 + otu 1x2 + tt 2x1
            tc.tile_pool(name="ps_st", bufs=2, space="PSUM") as ps_st,
            tc.tile_pool(name="ps_otu", bufs=1, space="PSUM") as ps_otu,
            tc.tile_pool(name="ps_t", bufs=2, space="PSUM") as ps_t,
        ):
            # ---------------- input DMAs ----------------
            # Two HWDGE queues (SP + scalar): split x across both so pair-0's
            # first QK chunk can start ~2x sooner; pair-0 weights ride the
            # scalar queue ahead of everything else.
            xt = []
            for ci in range(NCT):
                t_ = xw.tile([128, T], BF16, tag=f"xt{ci}", name=f"xt{ci}")
                q = nc.sync if ci % 2 == 0 else nc.scalar
                q.dma_start(out=t_, in_=xT[ci * 128:(ci + 1) * 128, :])
                xt.append(t_)
            wq_sb = [[None] * NCT for _ in range(3)]
            wk_sb = [[None] * NCT for _ in range(3)]
            for ci in range(NCT):
                tq_ = xw.tile([128, 128], BF16, tag=f"wq0_{ci}",
                              name=f"wq0_{ci}")
                nc.scalar.dma_start(out=tq_, in_=wq[0, ci])
                wq_sb[0][ci] = tq_
                tk_ = xw.tile([128, 128], BF16, tag=f"wk0_{ci}",
                              name=f"wk0_{ci}")
                nc.scalar.dma_start(out=tk_, in_=wk[0, ci])
                wk_sb[0][ci] = tk_
            wv_sb = []
            for ci in range(NCT):
                t_ = xw.tile([128, HL * HS], BF16, tag=f"wv{ci}", name=f"wv{ci}")
                nc.sync.dma_start(out=t_, in_=wv[ci])
                wv_sb.append(t_)
            for p in range(1, 3):
                for ci in range(NCT):
                    tq_ = xw.tile([128, 128], BF16, tag=f"wq{p}_{ci}",
                                  name=f"wq{p}_{ci}")
                    nc.sync.dma_start(out=tq_, in_=wq[p, ci])
                    wq_sb[p][ci] = tq_
                    tk_ = xw.tile([128, 128], BF16, tag=f"wk{p}_{ci}",
                                  name=f"wk{p}_{ci}")
                    nc.sync.dma_start(out=tk_, in_=wk[p, ci])
                    wk_sb[p][ci] = tk_
            wpt_sb = []
            for g in range(3):
                t_ = consts.tile([128, C], BF16, tag=f"wpt{g}", name=f"wpt{g}")
                nc.sync.dma_start(out=t_, in_=wpt[g])
                wpt_sb.append(t_)

            # persistent tensors
            vaug = consts.tile([128, NTT, HL, HS + 1], BF16)
            nc.gpsimd.memset(vaug[:, :, :, HS:HS + 1], 1.0)
            pairQ = [consts.tile([128, T], BF16, tag=f"pq{p}", name=f"pq{p}")
                     for p in range(3)]
            pairK = [consts.tile([128, T], BF16, tag=f"pk{p}", name=f"pk{p}")
                     for p in range(3)]
            otn = consts.tile([128, 3, T], BF16)
            ones_rows = consts.tile([128, HS + 1], F32)
            nc.gpsimd.memset(ones_rows, 1.0)

            import contextlib
            rep_ctx = (
                tc.For_i(0, repeat, 1,
                         hint_engines=(mybir.EngineType.PE,
                                       mybir.EngineType.DVE,
                                       mybir.EngineType.Activation,
                                       mybir.EngineType.SP,
                                       mybir.EngineType.Pool))
                if repeat > 1 else contextlib.nullcontext()
            )
            with rep_ctx:
                build_phases(nc, tc, consts, xw, ptp, small, ysbp,
                             ps_st, ps_otu, ps_t,
                             xt, wq_sb, wk_sb, wv_sb, wpt_sb,
                             vaug, pairQ, pairK, otn, ones_rows, y)

    nc.compile()
    return nc


def build_phases(nc, tc, consts, xw, ptp, small, ysbp,
                 ps_st, ps_otu, ps_t,
                 xt, wq_sb, wk_sb, wv_sb, wpt_sb,
                 vaug, pairQ, pairK, otn, ones_rows, y):
    # ---- filler unit builders (each issues one psum-group of PE work) ----
    def qk_unit(p, which, m):
        """Project one 512-col chunk of Q (which=0) or K (which=1) for pair p
        straight into pairQ/pairK (partition-aligned, no staging DMA)."""
        w = wq_sb[p] if which == 0 else wk_sb[p]
        dst = pairQ[p] if which == 0 else pairK[p]
        sl = slice(m * 512, (m + 1) * 512)
        ps = ps_t.tile([128, 512], F32, tag="tt", name=f"qk{p}_{which}_{m}")
        for ci in range(NCT):
            nc.tensor.matmul(ps, w[ci], xt[ci][:, sl],
                             start=(ci == 0), stop=(ci == NCT - 1))
        nc.vector.tensor_copy(out=dst[:, sl], in_=ps)

    def v_unit(tt):
        ps = ps_t.tile([128, HL * HS], F32, tag="tt", name=f"psv{tt}")
        for ci in range(NCT):
            nc.tensor.matmul(ps, xt[ci][:, tt * 128:(tt + 1) * 128], wv_sb[ci],
                             start=(ci == 0), stop=(ci == NCT - 1))
        nc.vector.tensor_copy(
            out=vaug[:, tt, :, 0:HS],
            in_=ps.rearrange("p (h d) -> p h d", h=HL),
        )

    def norm_rb(state):
        """Row-broadcast of the reciprocal rows on GPSIMD instead of the PE.
        The ucode reads literal partition 0, so hop the row down first with a
        tiny DMA; the whole chain is deferred a chunk, so latency is free."""
        p, m, otu_sb, rbs = state
        for e in range(2):
            stg = small.tile([1, 512], F32, tag="rstg", name=f"rs{p}_{m}_{e}")
            nc.sync.dma_start(out=stg, in_=otu_sb[HS:HS + 1, e, :])
            rb = small.tile([HS, 512], F32, tag="rbb", name=f"rb{p}_{m}_{e}")
            nc.gpsimd.partition_broadcast(rb, stg, channels=HS)
            rbs.append(rb)

    def norm_mul(state):
        p, m, otu_sb, rbs = state
        for e in range(2):
            otnorm = small.tile([HS, 512], BF16, tag="otnorm", name="otnorm")
            nc.vector.tensor_mul(out=otnorm, in0=otu_sb[0:HS, e, :],
                                 in1=rbs[e])
            nc.sync.dma_start(
                out=otn[64 * e:64 * e + HS, p, m * 512:(m + 1) * 512],
                in_=otnorm,
            )

    def proj_unit(tt, tail=False):
        y1 = ps_t.tile([128, 512], F32, tag="tt", name=f"y1_{tt}")
        y2 = ps_t.tile([128, 256], F32, tag="tt", name=f"y2_{tt}")
        for g in range(3):
            lhs = otn[:, g, tt * 128:(tt + 1) * 128]
            nc.tensor.matmul(y1, lhs, wpt_sb[g][:, 0:512],
                             start=(g == 0), stop=(g == 2))
            nc.tensor.matmul(y2, lhs, wpt_sb[g][:, 512:768],
                             start=(g == 0), stop=(g == 2))
        ysb = ysbp.tile([128, C], BF16, tag="ysb", name="ysb")
        # half-granular copy+DMA so the store overlaps the second copy; in
        # the tail ACT is idle (no more exps), so the second half-copy and
        # its DMA ride the scalar engine/queue in parallel with DVE.
        nc.vector.tensor_copy(out=ysb[:, 0:512], in_=y1)
        nc.sync.dma_start(out=y[tt * 128:(tt + 1) * 128, 0:512],
                          in_=ysb[:, 0:512])
        if tail:
            nc.scalar.copy(out=ysb[:, 512:768], in_=y2)
            nc.scalar.dma_start(out=y[tt * 128:(tt + 1) * 128, 512:768],
                                in_=ysb[:, 512:768])
        else:
            nc.vector.tensor_copy(out=ysb[:, 512:768], in_=y2)
            nc.sync.dma_start(out=y[tt * 128:(tt + 1) * 128, 512:768],
                              in_=ysb[:, 512:768])

    # ---- static filler schedule ----
    # fillers[(p, m)] = list of closures to interleave into chunk (p, m)'s
    # j-loop, one per j step starting at j=0.  Dependencies: V_j must be
    # issued before PV_j of any chunk that reads it; qk(p) chunks must
    # complete before pair p's attention reads them; proj(m) needs norm of
    # all three p at chunk m (p2's is issued one chunk earlier).
    fillers = {
        (0, 0): [lambda tt=t: v_unit(tt) for t in range(0, 4)]
                + [lambda: qk_unit(0, 0, 1), lambda: qk_unit(0, 1, 1)],
        (0, 1): [lambda tt=t: v_unit(tt) for t in range(4, 8)]
                + [lambda: qk_unit(0, 0, 2), lambda: qk_unit(0, 1, 2)],
        (0, 2): [lambda tt=t: v_unit(tt) for t in range(8, 12)]
                + [lambda: qk_unit(0, 0, 3), lambda: qk_unit(0, 1, 3)],
        (0, 3): [lambda tt=t: v_unit(tt) for t in range(12, 16)]
                + [lambda: qk_unit(1, 0, 0), lambda: qk_unit(1, 1, 0),
                   lambda: qk_unit(1, 0, 1), lambda: qk_unit(1, 1, 1)],
        (1, 0): [lambda: qk_unit(1, 0, 2), lambda: qk_unit(1, 1, 2)],
        (1, 1): [lambda: qk_unit(1, 0, 3), lambda: qk_unit(1, 1, 3),
                 lambda: qk_unit(2, 0, 0), lambda: qk_unit(2, 1, 0)],
        (1, 2): [lambda: qk_unit(2, 0, 1), lambda: qk_unit(2, 1, 1),
                 lambda: qk_unit(2, 0, 2), lambda: qk_unit(2, 1, 2)],
        (1, 3): [lambda: qk_unit(2, 0, 3), lambda: qk_unit(2, 1, 3)],
        (2, 0): [],
        (2, 1): [lambda tt=t: proj_unit(tt) for t in range(0, 4)],
        (2, 2): [lambda tt=t: proj_unit(tt) for t in range(4, 8)],
        (2, 3): [lambda tt=t: proj_unit(tt) for t in range(8, 12)],
    }

    # HAM warmup: dependency-free dummy matmuls stream while the input DMAs
    # land, so the PE clock gate is at 8/8 (2.4 GHz) when real work starts.
    wu = ps_t.tile([HS + 1, 64], F32, tag="tt", name="warmup")
    for _ in range(28):
        nc.tensor.matmul(wu, ones_rows, ones_rows[:, 0:64],
                         start=True, stop=True)

    # prefix: only pair-0 chunk-0 Q,K before attention starts
    qk_unit(0, 0, 0)
    qk_unit(0, 1, 0)

    pending_norm = []  # [(state, e), ...] deferred normalization units

    for p in range(3):
        for m in range(NTC):
            jmax = 4 * m + 3
            fl = list(fillers[(p, m)])
            # deferred norm units from the previous chunk go first
            fl = pending_norm + fl
            pending_norm = []
            fi = 0

            PV_LAG = 3 if jmax >= 3 else (2 if jmax >= 2 else 1)

            def pv(jj):
                ss = max(0, jj - 4 * m)
                for e in range(2):
                    nc.tensor.matmul(
                        otu_ps[:, e, 128 * ss:512],
                        vaug[:, jj, 2 * p + e, :],
                        pts[jj][:, e, 128 * ss:512],
                        start=(jj == 0), stop=(jj == jmax),
                        skip_group_check=True,
                    )

            otu_ps = ps_otu.tile([HS + 1, 2, 512], F32, tag="otu", name="otu")
            pts = []
            for j in range(jmax + 1):
                s0 = max(0, j - 4 * m)
                st = ps_st.tile([128, 2, 512], F32, tag="st", name="st")
                for e in range(2):
                    nc.tensor.matmul(
                        st[:, e, 128 * s0:512],
                        pairK[p][64 * e:64 * e + 64, j * 128:(j + 1) * 128],
                        pairQ[p][64 * e:64 * e + 64,
                                 m * 512 + 128 * s0:(m + 1) * 512],
                        start=True, stop=True,
                        tile_position=(64 * e, 0),
                    )
                pt = ptp.tile([128, 2, 512], BF16, tag="pt", name="pt")
                pts.append(pt)
                nc.scalar.activation(
                    out=pt[:, :, 128 * s0:512],
                    in_=st[:, :, 128 * s0:512],
                    func=mybir.ActivationFunctionType.Exp,
                    scale=SCALE,
                )
                if j >= 4 * m:
                    # zero below-diagonal of the diagonal subtile (both heads)
                    nc.gpsimd.affine_select(
                        out=pt[:, :, 128 * s0:128 * s0 + 128],
                        in_=pt[:, :, 128 * s0:128 * s0 + 128],
                        compare_op=mybir.AluOpType.is_ge,
                        fill=0.0, base=0,
                        pattern=[[0, 2], [1, 128]],
                        channel_multiplier=-1,
                    )
                # pace fillers evenly across the chunk (late units often
                # depend on fresh DVE/DMA results; bursting them early
                # stalls the PE on those chains)
                while fi < len(fl) and (j + 1) * len(fl) >= (fi + 1) * (jmax + 1):
                    fl[fi]()
                    fi += 1
                # PV lags the exp by PV_LAG j-steps for pipeline slack
                if j >= PV_LAG:
                    pv(j - PV_LAG)
            # drain leftover fillers, then the last PVs
            while fi < len(fl):
                fl[fi]()
                fi += 1
            for jj in range(max(0, jmax + 1 - PV_LAG), jmax + 1):
                pv(jj)
            # reciprocal straight from PSUM (tiny, unblocks rb early), then
            # stash the body to SBUF to free the psum for the next chunk;
            # rb/mul are deferred into the next chunk's filler slots.
            otu_sb = small.tile([HS + 1, 2, 512], F32, tag="otusb",
                                name="otusb")
            for e in range(2):
                nc.vector.reciprocal(out=otu_sb[HS:HS + 1, e, :],
                                     in_=otu_ps[HS:HS + 1, e, :])
            nc.vector.tensor_copy(out=otu_sb[0:HS], in_=otu_ps[0:HS])
            state = (p, m, otu_sb, [])
            pending_norm = [lambda s=state: norm_rb(s),
                            lambda s=state: norm_mul(s)]

    # tail: last chunk's norm + final projection row
    for u in pending_norm:
        u()
    for tt in range(12, 16):
        proj_unit(tt, tail=True)


_NC_CACHE = {}


def get_nc(repeat=1):
    key = repeat
    if key not in _NC_CACHE:
        nc = bacc.Bacc(
            "TRN2", target_bir_lowering=False, debug=False, num_devices=8
        )
        _NC_CACHE[key] = build_kernel(nc, repeat=repeat)
    return _NC_CACHE[key]


def make_in_maps(x, Wq, Wk, Wv, Wp):
    x = np.asarray(x, dtype=np.float32)
    Wq = np.asarray(Wq, dtype=np.float32)
    Wk = np.asarray(Wk, dtype=np.float32)
    Wv = np.asarray(Wv, dtype=np.float32)
    Wp = np.asarray(Wp, dtype=np.float32)
    bf = ml_dtypes.bfloat16
    in_maps = []
    for c in range(8):
        b = c // 2
        hs = HL * (c % 2)
        xT = np.ascontiguousarray(x[b].T).astype(bf)
        wq_ = np.empty((3, NCT, 128, 128), dtype=bf)
        wk_ = np.empty((3, NCT, 128, 128), dtype=bf)
        for p in range(3):
            sq = np.concatenate([Wq[hs + 2 * p], Wq[hs + 2 * p + 1]], axis=1)
            sk = np.concatenate([Wk[hs + 2 * p], Wk[hs + 2 * p + 1]], axis=1)
            for ci in range(NCT):
                wq_[p, ci] = sq[ci * 128:(ci + 1) * 128, :].astype(bf)
                wk_[p, ci] = sk[ci * 128:(ci + 1) * 128, :].astype(bf)
        wv_full = np.transpose(Wv[hs:hs + HL], (1, 0, 2)).reshape(C, HL * HS)
        wv_ = np.ascontiguousarray(
            wv_full.reshape(NCT, 128, HL * HS)
        ).astype(bf)
        wpt_ = np.ascontiguousarray(
            Wp[:, hs * HS:(hs + HL) * HS].T.reshape(3, 128, C)
        ).astype(bf)
        in_maps.append({"xT": xT, "wq": wq_, "wk": wk_, "wv": wv_,
                        "wpt": wpt_})
    return in_maps


def run(x, Wq, Wk, Wv, Wp, bp, trace=False):
    nc = get_nc()
    in_maps = make_in_maps(x, Wq, Wk, Wv, Wp)
    res = bass_utils.run_bass_kernel_spmd(
        nc, in_maps, core_ids=list(range(8)), trace=trace
    )
    y = np.zeros((B, T, C), dtype=np.float32)
    for c in range(8):
        y[c // 2] += np.asarray(res.results[c]["y"], dtype=np.float32)
    y += np.asarray(bp, dtype=np.float32)
    return y, res


def kernel(x, Wq, Wk, Wv, Wp, bp):
    y, _ = run(x, Wq, Wk, Wv, Wp, bp)
    return y


def make_runner(nc):
    """Build the sharded PJRT callable once. Returns (fn, prep, zeros,
    out_names, make_loop_fn)."""
    import jax
    from jax.experimental.shard_map import shard_map
    from jax.sharding import Mesh, PartitionSpec, NamedSharding
    from concourse import mybir as _mybir
    from concourse.bass2jax import (
        _bass_exec_p, install_neuronx_cc_hook, partition_id_tensor,
    )

    install_neuronx_cc_hook()
    n_cores = 8
    partition_name = (
        nc.partition_id_tensor.name if nc.partition_id_tensor else None
    )
    in_names, out_names, out_avals = [], [], []
    for alloc in nc.m.functions[0].allocations:
        if not isinstance(alloc, _mybir.MemoryLocationSet):
            continue
        name = alloc.memorylocations[0].name
        if alloc.kind == "ExternalInput":
            if name != partition_name:
                in_names.append(name)
        elif alloc.kind == "ExternalOutput":
            out_names.append(name)
            out_avals.append(
                jax.core.ShapedArray(
                    tuple(alloc.tensor_shape), _mybir.dt.np(alloc.dtype)
                )
            )
    n_params = len(in_names)
    n_outs = len(out_avals)
    all_in_names = in_names + out_names
    if partition_name is not None:
        all_in_names.append(partition_name)

    def _body(*args):
        operands = list(args)
        if partition_name is not None:
            operands.append(partition_id_tensor())
        outs = _bass_exec_p.bind(
            *operands,
            out_avals=tuple(out_avals),
            in_names=tuple(all_in_names),
            out_names=tuple(out_names),
            lowering_input_output_aliases=(),
            sim_require_finite=True,
            sim_require_nnan=True,
            nc=nc,
        )
        return tuple(outs)

    devices = jax.devices()[:n_cores]
    mesh = Mesh(np.array(devices), ("core",))
    sharded = jax.jit(
        shard_map(
            _body, mesh=mesh,
            in_specs=(PartitionSpec("core"),) * (n_params + n_outs),
            out_specs=(PartitionSpec("core"),) * n_outs,
            check_rep=False,
        ),
        donate_argnums=tuple(range(n_params, n_params + n_outs)),
        keep_unused=True,
    )
    shd = NamedSharding(mesh, PartitionSpec("core"))

    def prep(in_maps):
        return [
            jax.device_put(
                np.concatenate([in_maps[c][nm] for c in range(n_cores)], axis=0),
                shd,
            )
            for nm in in_names
        ]

    def zeros():
        return [
            jax.device_put(
                np.zeros((n_cores * a.shape[0], *a.shape[1:]), a.dtype), shd
            )
            for a in out_avals
        ]

    def fn(dev_inputs, dev_zeros):
        outs = sharded(*dev_inputs, *dev_zeros)
        jax.block_until_ready(outs)
        return outs

    def make_loop_fn(n_iters):
        def _body_n(*args):
            ins = args[:n_params]
            carry = tuple(args[n_params:])

            def step(i, carry):
                operands = list(ins) + list(carry)
                if partition_name is not None:
                    operands.append(partition_id_tensor())
                outs = _bass_exec_p.bind(
                    *operands,
                    out_avals=tuple(out_avals),
                    in_names=tuple(all_in_names),
                    out_names=tuple(out_names),
                    lowering_input_output_aliases=(),
                    sim_require_finite=True,
                    sim_require_nnan=True,
                    nc=nc,
                )
                return tuple(outs)

            return jax.lax.fori_loop(0, n_iters, step, carry)

        looped = jax.jit(
            shard_map(
                _body_n, mesh=mesh,
                in_specs=(PartitionSpec("core"),) * (n_params + n_outs),
                out_specs=(PartitionSpec("core"),) * n_outs,
                check_rep=False,
            ),
            donate_argnums=tuple(range(n_params, n_params + n_outs)),
            keep_unused=True,
        )

        def run_n(dev_inputs, dev_zeros):
            outs = looped(*dev_inputs, *dev_zeros)
            jax.block_until_ready(outs)
            return outs

        return run_n

    return fn, prep, zeros, out_names, make_loop_fn

